# revision 39
# baseline (speedup 1.0000x reference)
"""ClusterGCN 3-layer GNN on 8 TRN2 NeuronCores.  5.38ms -> 1.52ms.

Design (the gather-descriptor pipe is the roofline here):
- Nodes (destinations) sharded across 8 cores (6250 each); weights replicated.
  The node-feature table is replicated in every core's HBM (bf16) in a
  PERMUTED row order (group-major, then core, then local row) so the
  between-layer AllGather runs as 7 contiguous group slices, dispatched with
  a 2-group lag so they never stall the Pool engine's gather stream.
- Per core, non-self edges are grouped by dest window (128 dests) and source
  half (gather idx must fit int16), padded to a uniform static chunk grid
  (CH=18 chunks/window).  Self-loops are NOT gathered: their term is one
  identity matmul per window from a resident node-major activation buffer.
- dma_gather (Q7 SWDGE) pulls source rows (256B) into SBUF in edge order.
  Gathers round-robin over 4 SWDGE queues (num_swdge_queues=4) - the single
  largest win: the sw-dynamic descriptor pipe is the end-to-end bottleneck
  (~6ns/packet at 4 queues vs ~11ns at 1).
- Segment-sum = TensorE matmul against a HOST-PRECOMPUTED 0/1 routing matrix
  S (fp8, streamed from HBM, 40us/layer of cheap sequential DMA): aggT[f,d]
  += sum_e msg[e,f]*S[e,d] in PSUM.  This removed ~2900 per-chunk DVE
  tensor_scalar builds (~1.5us each on HW - 4.9ms of DVE!).  deg_inv is
  folded into the PSUM->SBUF cast (one DVE tensor_tensor per window against
  a replicated deg_inv table).
- Dense phase per window: hp[d,n] = aggT.T@W_out + xT.T@W_root, relu on
  ScalarE.  Root-path input xT stays feature-major RESIDENT in SBUF across
  layers; each layer also computes hT[n,d] via swapped-operand matmuls and
  relus it into the next layer's xT buffer (no transpose DMAs, no reloads).
  PE is software-pipelined: window w's aggregation runs while w-1's dense
  matmuls wait on the DVE fold.
- Final layer: relu'd logits accumulate in SBUF f32; log_softmax is batched
  per group (Exp+accum / Ln / broadcast-subtract / strided DMA out) with no
  max-subtraction (logits bounded, fp32 exp safe).
"""
import sys
sys.path.insert(0, "/opt/trn_rl_repo")
import os
import numpy as np
import ml_dtypes

import concourse.bacc as bacc
import concourse.bass as bass
import concourse.mybir as mybir
import concourse.tile as tile
from concourse.bass_utils import run_bass_kernel_spmd

NCORES = 8
BF16 = ml_dtypes.bfloat16
FP8 = ml_dtypes.float8_e4m3fn
LAST_EXEC_NS = None

N = 50000
C = N // NCORES              # 6250 nodes per core
WN = (C + 127) // 128        # 49 dest windows per core
G = 7                        # windows per gather/collective group
NG = WN // G                 # 7 groups
GR = G * 128                 # 896 rows per (core, group)
ROWS = WN * 128              # 6272 padded rows per core
NT = NCORES * ROWS           # 50176 permuted table rows
GROUP_ROWS = NCORES * GR     # 7168 table rows per group slice
HALF_T = NT // 2             # 25088 (int16-safe half split)


def _wrap_idx(idx16: np.ndarray) -> np.ndarray:
    """[n] int16 -> [128, n/16] wrapped (idx i at [i%16, i//16]), replicated
    8x down partitions for the 8 Q7 cores."""
    w = idx16.reshape(-1, 16).T.astype(np.int16)
    return np.tile(w, (8, 1))


def _preprocess(x, edge_index):
    # self-loops are NOT materialized as edges (handled by an identity matmul
    # from the resident node-major buffer); degree still counts them.
    src = edge_index[0].astype(np.int64)
    dst = edge_index[1].astype(np.int64)
    deg = (np.bincount(dst, minlength=N) + 1).astype(np.float32)
    dinv = 1.0 / np.maximum(deg, 1.0)

    # node s -> permuted table row (group-major, core, local offset)
    s_all = np.arange(N)
    kk = s_all // C
    ll = s_all - kk * C
    gg = ll // GR
    oo = ll - gg * GR
    trow = gg * (NCORES * GR) + kk * GR + oo          # [N]

    kd = dst // C
    ld = dst - kd * C
    win = ld >> 7
    dl = ld & 127
    r = trow[src]
    hi = (r >= HALF_T).astype(np.int64)
    rel = (r - hi * HALF_T).astype(np.int64)
    key = (kd * WN + win) * 2 + hi
    order = np.argsort(key, kind="stable")
    rel_s, dl_s, key_s = rel[order], dl[order], key[order]
    counts = np.bincount(key, minlength=NCORES * WN * 2)
    starts = np.zeros(NCORES * WN * 2 + 1, np.int64)
    np.cumsum(counts, out=starts[1:])
    NL = int(np.ceil(counts[0::2].max() / 128))
    NH = int(np.ceil(counts[1::2].max() / 128))
    CH = NL + NH

    per_core = []
    for k in range(NCORES):
        lo_idx = np.zeros((WN, NL * 128), np.int16)
        hi_idx = np.zeros((WN, NH * 128), np.int16)
        Sh = np.zeros((128, WN * CH * 128), FP8)
        for w in range(WN):
            b = (k * WN + w) * 2
            s0, s1, s2 = starts[b], starts[b + 1], starts[b + 2]
            nlo, nhi = s1 - s0, s2 - s1
            lo_idx[w, :nlo] = rel_s[s0:s1].astype(np.int16)
            hi_idx[w, :nhi] = rel_s[s1:s2].astype(np.int16)
            p = np.arange(nlo)
            Sh[p % 128, (w * CH + p // 128) * 128 + dl_s[s0:s1]] = 1.0
            p = np.arange(nhi)
            Sh[p % 128, (w * CH + NL + p // 128) * 128 + dl_s[s1:s2]] = 1.0
        gcols = []
        for g in range(NG):
            gcols.append(_wrap_idx(lo_idx[g * G:(g + 1) * G].reshape(-1)))
            gcols.append(_wrap_idx(hi_idx[g * G:(g + 1) * G].reshape(-1)))
        gidx = np.concatenate(gcols, axis=1)          # [128, WN*CH*8]
        dv = np.zeros(ROWS, np.float32)
        dv[:C] = dinv[k * C:(k + 1) * C]
        DINV = np.tile(dv.astype(BF16), (128, 1))     # [128, ROWS]
        per_core.append((gidx, Sh, DINV))

    # permuted full table of x (bf16)
    tbl1 = np.zeros((NT, x.shape[1]), BF16)
    tbl1[trow] = x.astype(BF16)
    return per_core, tbl1, dict(NL=NL, NH=NH, CH=CH)


IDENT = np.eye(128, dtype=BF16)


def _build(dims, d_in, d_h, d_out, use_bias):
    NL, NH, CH = dims["NL"], dims["NH"], dims["CH"]
    f32, bf, i16 = mybir.dt.float32, mybir.dt.bfloat16, mybir.dt.int16
    f8 = mybir.dt.float8e4
    AF = mybir.ActivationFunctionType
    OP = mybir.AluOpType

    nc = bacc.Bacc("TRN2", num_devices=NCORES,
                   num_swdge_queues=int(os.environ.get("GCN_NQ", "4")))

    tbl1 = nc.dram_tensor("tbl1", [NT, d_in], bf, kind="ExternalInput")
    gidx_h = nc.dram_tensor("gidx", [128, WN * CH * 8], i16, kind="ExternalInput")
    sh_h = nc.dram_tensor("sh", [128, WN * CH * 128], f8, kind="ExternalInput")
    dinv_h = nc.dram_tensor("dinv", [128, ROWS], bf, kind="ExternalInput")
    xt1_h = nc.dram_tensor("xt1", [128, ROWS], bf, kind="ExternalInput")
    xn1_h = nc.dram_tensor("xn1", [128, ROWS], bf, kind="ExternalInput")
    ident_h = nc.dram_tensor("ident", [128, 128], bf, kind="ExternalInput")
    w_h = {}
    for nm, shp in [("w1o", [d_in, d_h]), ("w1r", [d_in, d_h]),
                    ("w2o", [d_h, d_h]), ("w2r", [d_h, d_h]),
                    ("w3o", [d_h, d_out]), ("w3r", [d_h, d_out])]:
        w_h[nm] = nc.dram_tensor(nm, shp, bf, kind="ExternalInput")
    bias_h = {}
    if use_bias:
        for nm, dd in [("b1", d_h), ("b2", d_h), ("b3", d_out)]:
            bias_h[nm] = nc.dram_tensor(nm, [128, dd], f32, kind="ExternalInput")
        for nm in ("b1c", "b2c"):
            bias_h[nm] = nc.dram_tensor(nm, [128, 1], f32, kind="ExternalInput")

    out_h = nc.dram_tensor("out", [ROWS, d_out], f32, kind="ExternalOutput")
    tbl2 = nc.dram_tensor("tbl2", [NT, d_h], bf, addr_space="Shared")
    tbl3 = nc.dram_tensor("tbl3", [NT, d_h], bf, addr_space="Shared")
    # per-group collective staging (separate tensors avoid false deps)
    hbg = {(L, g): nc.dram_tensor(f"hb{L}_{g}", [GR, d_h], bf)
           for L in (1, 2) for g in range(NG)}

    with tile.TileContext(nc, num_cores=NCORES) as tc:
        with (
            tc.tile_pool(name="const", bufs=1) as const,
            tc.tile_pool(name="msgp", bufs=3) as msgp,
            tc.tile_pool(name="sp", bufs=2) as sp,
            tc.tile_pool(name="wk", bufs=3) as wk,
            tc.tile_pool(name="sm", bufs=1) as sm,
            tc.tile_pool(name="ps", bufs=2, space="PSUM") as ps,
        ):
            gidx_t = const.tile([128, WN * CH * 8], i16)
            nc.sync.dma_start(gidx_t[:], gidx_h[:])
            dinv_t = const.tile([128, ROWS], bf)
            nc.sync.dma_start(dinv_t[:], dinv_h[:])
            xta = const.tile([128, ROWS], bf)
            nc.sync.dma_start(xta[:], xt1_h[:])
            xtb = const.tile([128, ROWS], bf)
            # node-major activations of the current layer's input (for the
            # self-loop term and the collective staging)
            xn = const.tile([128, ROWS], bf)
            nc.sync.dma_start(xn[:], xn1_h[:])
            ident_t = const.tile([128, 128], bf)
            nc.sync.dma_start(ident_t[:], ident_h[:])
            w_t = {}
            for nm, hh in w_h.items():
                w_t[nm] = const.tile(list(hh.shape), bf, name=f"wt_{nm}")
                nc.sync.dma_start(w_t[nm][:], hh[:])
            b_t = {}
            for nm, hh in bias_h.items():
                b_t[nm] = const.tile(list(hh.shape), f32, name=f"bt_{nm}")
                nc.sync.dma_start(b_t[nm][:], hh[:])
            h3_t = sm.tile([128, WN, d_out], f32)
            ssum_t = sm.tile([128, WN], f32)
            ex_t = sm.tile([128, d_out], f32)
            lns_t = sm.tile([128, WN], f32)
            fin_state = {"ndone": 0, "gfin": 0}

            def layer(L, tbl_in, xt_in, xt_out, wo, wr, bname, dd):
                last = L == 3
                pend = None  # (w, g, j, aggT_s)
                ccq = []   # groups whose collective slice is pending dispatch
                expq = []  # windows awaiting their Exp pass (layer 3)

                def dispatch_cc(g):
                    tbl_next = tbl2 if L == 1 else tbl3
                    nc.gpsimd.collective_compute(
                        "AllGather", mybir.AluOpType.bypass,
                        replica_groups=[list(range(NCORES))],
                        ins=[hbg[(L, g)][:, :]],
                        outs=[tbl_next[g * GROUP_ROWS:(g + 1) * GROUP_ROWS, :]],
                    )

                def flush_exp(final=False):
                    # per-group log-softmax finale: exp+accum each pending
                    # window, then Ln/subtract/store for complete groups
                    while expq:
                        w = expq.pop(0)
                        nc.scalar.activation(ex_t[:], h3_t[:, w, :], AF.Exp,
                                             accum_out=ssum_t[:, w:w + 1])
                        fin_state["ndone"] = w + 1
                    gdone = fin_state["ndone"] // G if not final else NG
                    while fin_state["gfin"] < gdone:
                        gg = fin_state["gfin"]
                        w0, w1 = gg * G, (gg + 1) * G
                        nc.scalar.activation(lns_t[:, w0:w1],
                                             ssum_t[:, w0:w1], AF.Ln)
                        nc.vector.tensor_tensor(
                            h3_t[:, w0:w1, :], h3_t[:, w0:w1, :],
                            lns_t[:, w0:w1].unsqueeze(2).broadcast_to(
                                [128, G, d_out]),
                            OP.subtract)
                        nc.sync.dma_start(
                            out_h[:].rearrange("(w d) n -> d w n", d=128)
                            [:, w0:w1, :],
                            h3_t[:, w0:w1, :])
                        fin_state["gfin"] = gg + 1

                def dense(w, g, j, aggT_s):
                    hp = ps.tile([128, dd], f32, tag="hp")
                    nc.tensor.matmul(hp[:], aggT_s[:], wo[:], start=True, stop=False)
                    nc.tensor.matmul(hp[:], xt_in[:, w * 128:(w + 1) * 128],
                                     wr[:], start=False, stop=True)
                    if bname is not None:
                        nc.vector.tensor_add(hp[:], hp[:], b_t[bname][:, 0:dd])
                    if not last:
                        nc.scalar.activation(
                            xn[:, w * 128:(w + 1) * 128], hp[:], AF.Relu)
                        nc.sync.dma_start(
                            hbg[(L, g)][j * 128:(j + 1) * 128, :],
                            xn[:, w * 128:(w + 1) * 128])
                        htp = ps.tile([128, 128], f32, tag="htp")
                        nc.tensor.matmul(htp[:], wo[:], aggT_s[:],
                                         start=True, stop=False)
                        nc.tensor.matmul(htp[:], wr[:],
                                         xt_in[:, w * 128:(w + 1) * 128],
                                         start=False, stop=True)
                        if bname is not None:
                            nc.scalar.activation(
                                xt_out[:, w * 128:(w + 1) * 128], htp[:],
                                AF.Relu, bias=b_t[bname + "c"][:, 0:1])
                        else:
                            nc.scalar.activation(
                                xt_out[:, w * 128:(w + 1) * 128], htp[:], AF.Relu)
                    else:
                        nc.scalar.activation(
                            h3_t[:, w, :], hp[:], AF.Relu)
                        expq.append(w)
                    if not last and j == G - 1:
                        ccq.append(g)

                for g in range(NG):
                    msg = msgp.tile([128, G * CH, d_h], bf, tag="msg")
                    col0 = (g * CH * 8) * G
                    nlo16, nhi16 = G * NL * 8, G * NH * 8
                    nq = int(os.environ.get("GCN_NQ", "2"))
                    sp_flag = bool(int(os.environ.get("GCN_SP", "0")))
                    nc.gpsimd.dma_gather(
                        msg[:, 0:G * NL, :], tbl_in[0:HALF_T, :],
                        gidx_t[:, col0:col0 + nlo16],
                        G * NL * 128, G * NL * 128, d_h,
                        single_packet=sp_flag,
                        queue_num=(2 * g) % nq,
                    )
                    nc.gpsimd.dma_gather(
                        msg[:, G * NL:G * CH, :], tbl_in[HALF_T:NT, :],
                        gidx_t[:, col0 + nlo16:col0 + nlo16 + nhi16],
                        G * NH * 128, G * NH * 128, d_h,
                        single_packet=sp_flag,
                        queue_num=(2 * g + 1) % nq,
                    )
                    s_t = sp.tile([128, G * CH * 128], f8, tag="s_t")
                    nc.sync.dma_start(
                        s_t[:], sh_h[:, g * G * CH * 128:(g + 1) * G * CH * 128])
                    while ccq and ccq[0] <= g - 2:
                        dispatch_cc(ccq.pop(0))
                    if last:
                        flush_exp()
                    for j in range(G):
                        w = g * G + j
                        aggT = ps.tile([128, 128], f32, tag="aggT")
                        # self-loop term: aggT[f,d] = xn[d,f] via identity
                        nc.tensor.matmul(
                            aggT[:], xn[:, w * 128:(w + 1) * 128], ident_t[:],
                            start=True, stop=False,
                        )
                        for c in range(CH):
                            pos = j * NL + c if c < NL else G * NL + j * NH + (c - NL)
                            nc.tensor.matmul(
                                aggT[:], msg[:, pos, :],
                                s_t[:, (j * CH + c) * 128:(j * CH + c + 1) * 128],
                                start=False, stop=(c == CH - 1),
                            )
                        aggT_s = wk.tile([128, 128], bf, tag="aggs")
                        nc.vector.tensor_tensor(
                            aggT_s[:], aggT[:],
                            dinv_t[:, w * 128:(w + 1) * 128], OP.mult)
                        if pend is not None:
                            dense(*pend)
                        pend = (w, g, j, aggT_s)
                dense(*pend)
                while ccq:
                    dispatch_cc(ccq.pop(0))
                if last:
                    flush_exp(final=True)

            layer(1, tbl1, xta, xtb, w_t["w1o"], w_t["w1r"],
                  "b1" if use_bias else None, d_h)
            layer(2, tbl2, xtb, xta, w_t["w2o"], w_t["w2r"],
                  "b2" if use_bias else None, d_h)
            layer(3, tbl3, xta, None, w_t["w3o"], w_t["w3r"],
                  "b3" if use_bias else None, d_out)



    nc.compile()
    return nc


def kernel(x, edge_index, W1_out, b1, W1_root, W2_out, b2, W2_root,
           W3_out, b3, W3_root):
    global LAST_EXEC_NS
    x = np.asarray(x, np.float32)
    edge_index = np.asarray(edge_index)
    d_in = x.shape[1]
    d_h = W1_out.shape[1]
    d_out = W3_out.shape[1]
    per_core, tbl1, dims = _preprocess(x, edge_index)
    use_bias = bool(np.any(b1) or np.any(b2) or np.any(b3))

    nc = _build(dims, d_in, d_h, d_out, use_bias)

    in_maps = []
    for k in range(NCORES):
        gidx, Sh, DINV = per_core[k]
        xt1 = np.zeros((128, ROWS), BF16)
        xt1[:, :C] = x[k * C:(k + 1) * C].T.astype(BF16)
        xn1 = np.zeros((128, ROWS), BF16)
        xk = x[k * C:(k + 1) * C].astype(BF16)          # [C, 128]
        xn1[:, :] = np.pad(xk, ((0, ROWS - C), (0, 0))).reshape(
            WN, 128, 128).transpose(1, 0, 2).reshape(128, ROWS)
        m = {
            "tbl1": tbl1,
            "gidx": gidx, "sh": Sh, "dinv": DINV, "xt1": xt1,
            "xn1": xn1, "ident": IDENT,
            "w1o": np.asarray(W1_out, np.float32).astype(BF16),
            "w1r": np.asarray(W1_root, np.float32).astype(BF16),
            "w2o": np.asarray(W2_out, np.float32).astype(BF16),
            "w2r": np.asarray(W2_root, np.float32).astype(BF16),
            "w3o": np.asarray(W3_out, np.float32).astype(BF16),
            "w3r": np.asarray(W3_root, np.float32).astype(BF16),
        }
        if use_bias:
            m["b1"] = np.tile(np.asarray(b1, np.float32), (128, 1))
            m["b2"] = np.tile(np.asarray(b2, np.float32), (128, 1))
            m["b3"] = np.tile(np.asarray(b3, np.float32), (128, 1))
            m["b1c"] = np.asarray(b1, np.float32).reshape(128, 1)
            m["b2c"] = np.asarray(b2, np.float32).reshape(128, 1)
        in_maps.append(m)

    trace = bool(int(os.environ.get("BASS_GCN_TRACE", "1")))
    if trace:
        try:
            # NTFF profiling under axon needs this hook module; without it
            # run_bass_kernel_spmd(trace=True) raises instead of degrading.
            from antenv.axon_hooks import get_axon_ntff_profile_hook  # noqa
        except ImportError:
            trace = False
    try:
        res = run_bass_kernel_spmd(nc, in_maps, core_ids=list(range(NCORES)),
                                   trace=trace)
    except Exception:
        if not trace:
            raise
        res = run_bass_kernel_spmd(nc, in_maps, core_ids=list(range(NCORES)),
                                   trace=False)
    LAST_EXEC_NS = res.exec_time_ns
    out = np.concatenate([res.results[k]["out"][:C] for k in range(NCORES)], axis=0)
    return out.astype(np.float32)


# revision 41
# speedup vs baseline: 1.0558x; 1.0558x over previous
"""ClusterGCN 3-layer GNN on 8 TRN2 NeuronCores.  5.38ms -> 1.52ms.

Design (the gather-descriptor pipe is the roofline here):
- Nodes (destinations) sharded across 8 cores (6250 each); weights replicated.
  The node-feature table is replicated in every core's HBM (bf16) in a
  PERMUTED row order (group-major, then core, then local row) so the
  between-layer AllGather runs as 7 contiguous group slices, dispatched with
  a 2-group lag so they never stall the Pool engine's gather stream.
- Per core, non-self edges are grouped by dest window (128 dests) and source
  half (gather idx must fit int16), padded to a uniform static chunk grid
  (CH=18 chunks/window).  Self-loops are NOT gathered: their term is one
  identity matmul per window from a resident node-major activation buffer.
- dma_gather (Q7 SWDGE) pulls source rows (256B) into SBUF in edge order.
  Gathers round-robin over 4 SWDGE queues (num_swdge_queues=4) - the single
  largest win: the sw-dynamic descriptor pipe is the end-to-end bottleneck
  (~6ns/packet at 4 queues vs ~11ns at 1).
- Segment-sum = TensorE matmul against a HOST-PRECOMPUTED 0/1 routing matrix
  S (fp8, streamed from HBM, 40us/layer of cheap sequential DMA): aggT[f,d]
  += sum_e msg[e,f]*S[e,d] in PSUM.  This removed ~2900 per-chunk DVE
  tensor_scalar builds (~1.5us each on HW - 4.9ms of DVE!).  deg_inv is
  folded into the PSUM->SBUF cast (one DVE tensor_tensor per window against
  a replicated deg_inv table).
- Dense phase per window: hp[d,n] = aggT.T@W_out + xT.T@W_root, relu on
  ScalarE.  Root-path input xT stays feature-major RESIDENT in SBUF across
  layers; each layer also computes hT[n,d] via swapped-operand matmuls and
  relus it into the next layer's xT buffer (no transpose DMAs, no reloads).
  PE is software-pipelined: window w's aggregation runs while w-1's dense
  matmuls wait on the DVE fold.
- Final layer: relu'd logits accumulate in SBUF f32; log_softmax is batched
  per group (Exp+accum / Ln / broadcast-subtract / strided DMA out) with no
  max-subtraction (logits bounded, fp32 exp safe).
"""
import sys
sys.path.insert(0, "/opt/trn_rl_repo")
import os
import numpy as np
import ml_dtypes

import concourse.bacc as bacc
import concourse.bass as bass
import concourse.mybir as mybir
import concourse.tile as tile
from concourse.bass_utils import run_bass_kernel_spmd

NCORES = 8
BF16 = ml_dtypes.bfloat16
FP8 = ml_dtypes.float8_e4m3fn
LAST_EXEC_NS = None

N = 50000
C = N // NCORES              # 6250 nodes per core
WN = (C + 127) // 128        # 49 dest windows per core
G = 7                        # windows per gather/collective group
NG = WN // G                 # 7 groups
GR = G * 128                 # 896 rows per (core, group)
ROWS = WN * 128              # 6272 padded rows per core
NT = NCORES * ROWS           # 50176 permuted table rows
GROUP_ROWS = NCORES * GR     # 7168 table rows per group slice
HALF_T = NT // 2             # 25088 (int16-safe half split)


def _wrap_idx(idx16: np.ndarray) -> np.ndarray:
    """[n] int16 -> [128, n/16] wrapped (idx i at [i%16, i//16]), replicated
    8x down partitions for the 8 Q7 cores."""
    w = idx16.reshape(-1, 16).T.astype(np.int16)
    return np.tile(w, (8, 1))


def _preprocess(x, edge_index):
    # self-loops are NOT materialized as edges (handled by an identity matmul
    # from the resident node-major buffer); degree still counts them.
    src = edge_index[0].astype(np.int64)
    dst = edge_index[1].astype(np.int64)
    deg = (np.bincount(dst, minlength=N) + 1).astype(np.float32)
    dinv = 1.0 / np.maximum(deg, 1.0)

    # node s -> permuted table row (group-major, core, local offset)
    s_all = np.arange(N)
    kk = s_all // C
    ll = s_all - kk * C
    gg = ll // GR
    oo = ll - gg * GR
    trow = gg * (NCORES * GR) + kk * GR + oo          # [N]

    kd = dst // C
    ld = dst - kd * C
    win = ld >> 7
    dl = ld & 127
    r = trow[src]
    hi = (r >= HALF_T).astype(np.int64)
    rel = (r - hi * HALF_T).astype(np.int64)
    key = (kd * WN + win) * 2 + hi
    order = np.argsort(key, kind="stable")
    rel_s, dl_s, key_s = rel[order], dl[order], key[order]
    counts = np.bincount(key, minlength=NCORES * WN * 2)
    starts = np.zeros(NCORES * WN * 2 + 1, np.int64)
    np.cumsum(counts, out=starts[1:])
    NL = int(np.ceil(counts[0::2].max() / 128))
    NH = int(np.ceil(counts[1::2].max() / 128))
    CH = NL + NH

    per_core = []
    for k in range(NCORES):
        lo_idx = np.zeros((WN, NL * 128), np.int16)
        hi_idx = np.zeros((WN, NH * 128), np.int16)
        Sh = np.zeros((128, WN * CH * 128), FP8)
        for w in range(WN):
            b = (k * WN + w) * 2
            s0, s1, s2 = starts[b], starts[b + 1], starts[b + 2]
            nlo, nhi = s1 - s0, s2 - s1
            lo_idx[w, :nlo] = rel_s[s0:s1].astype(np.int16)
            hi_idx[w, :nhi] = rel_s[s1:s2].astype(np.int16)
            p = np.arange(nlo)
            Sh[p % 128, (w * CH + p // 128) * 128 + dl_s[s0:s1]] = 1.0
            p = np.arange(nhi)
            Sh[p % 128, (w * CH + NL + p // 128) * 128 + dl_s[s1:s2]] = 1.0
        gcols = []
        for g in range(NG):
            gcols.append(_wrap_idx(lo_idx[g * G:(g + 1) * G].reshape(-1)))
            gcols.append(_wrap_idx(hi_idx[g * G:(g + 1) * G].reshape(-1)))
        gidx = np.concatenate(gcols, axis=1)          # [128, WN*CH*8]
        dv = np.zeros(ROWS, np.float32)
        dv[:C] = dinv[k * C:(k + 1) * C]
        DINV = np.tile(dv.astype(BF16), (128, 1))     # [128, ROWS]
        per_core.append((gidx, Sh, DINV))

    # permuted full table of x (bf16)
    tbl1 = np.zeros((NT, x.shape[1]), BF16)
    tbl1[trow] = x.astype(BF16)
    return per_core, tbl1, dict(NL=NL, NH=NH, CH=CH)


IDENT = np.eye(128, dtype=BF16)


def _build(dims, d_in, d_h, d_out, use_bias):
    NL, NH, CH = dims["NL"], dims["NH"], dims["CH"]
    f32, bf, i16 = mybir.dt.float32, mybir.dt.bfloat16, mybir.dt.int16
    f8 = mybir.dt.float8e4
    AF = mybir.ActivationFunctionType
    OP = mybir.AluOpType

    nc = bacc.Bacc("TRN2", num_devices=NCORES,
                   num_swdge_queues=int(os.environ.get("GCN_NQ", "4")))

    tbl1 = nc.dram_tensor("tbl1", [NT, d_in], bf, kind="ExternalInput")
    gidx_h = nc.dram_tensor("gidx", [128, WN * CH * 8], i16, kind="ExternalInput")
    sh_h = nc.dram_tensor("sh", [128, WN * CH * 128], f8, kind="ExternalInput")
    dinv_h = nc.dram_tensor("dinv", [128, ROWS], bf, kind="ExternalInput")
    xt1_h = nc.dram_tensor("xt1", [128, ROWS], bf, kind="ExternalInput")
    xn1_h = nc.dram_tensor("xn1", [128, ROWS], bf, kind="ExternalInput")
    ident_h = nc.dram_tensor("ident", [128, 128], bf, kind="ExternalInput")
    w_h = {}
    for nm, shp in [("w1o", [d_in, d_h]), ("w1r", [d_in, d_h]),
                    ("w2o", [d_h, d_h]), ("w2r", [d_h, d_h]),
                    ("w3o", [d_h, d_out]), ("w3r", [d_h, d_out])]:
        w_h[nm] = nc.dram_tensor(nm, shp, bf, kind="ExternalInput")
    bias_h = {}
    if use_bias:
        for nm, dd in [("b1", d_h), ("b2", d_h), ("b3", d_out)]:
            bias_h[nm] = nc.dram_tensor(nm, [128, dd], f32, kind="ExternalInput")
        for nm in ("b1c", "b2c"):
            bias_h[nm] = nc.dram_tensor(nm, [128, 1], f32, kind="ExternalInput")

    out_h = nc.dram_tensor("out", [ROWS, d_out], f32, kind="ExternalOutput")
    tbl2 = nc.dram_tensor("tbl2", [NT, d_h], bf, addr_space="Shared")
    tbl3 = nc.dram_tensor("tbl3", [NT, d_h], bf, addr_space="Shared")
    # per-group collective staging (separate tensors avoid false deps)
    hbg = {(L, g): nc.dram_tensor(f"hb{L}_{g}", [GR, d_h], bf)
           for L in (1, 2) for g in range(NG)}

    with tile.TileContext(nc, num_cores=NCORES) as tc:
        with (
            tc.tile_pool(name="const", bufs=1) as const,
            tc.tile_pool(name="msgp", bufs=3) as msgp,
            tc.tile_pool(name="sp", bufs=2) as sp,
            tc.tile_pool(name="wk", bufs=3) as wk,
            tc.tile_pool(name="sm", bufs=1) as sm,
            tc.tile_pool(name="ps", bufs=2, space="PSUM") as ps,
        ):
            gidx_t = const.tile([128, WN * CH * 8], i16)
            # group-0 columns first so the first gather launches ASAP
            nc.sync.dma_start(gidx_t[:, 0:G * CH * 8], gidx_h[:, 0:G * CH * 8])
            nc.sync.dma_start(gidx_t[:, G * CH * 8:], gidx_h[:, G * CH * 8:])
            dinv_t = const.tile([128, ROWS], bf)
            nc.sync.dma_start(dinv_t[:], dinv_h[:])
            xta = const.tile([128, ROWS], bf)
            nc.sync.dma_start(xta[:], xt1_h[:])
            xtb = const.tile([128, ROWS], bf)
            # node-major activations of the current layer's input (for the
            # self-loop term and the collective staging)
            xn = const.tile([128, ROWS], bf)
            nc.sync.dma_start(xn[:], xn1_h[:])
            ident_t = const.tile([128, 128], bf)
            nc.sync.dma_start(ident_t[:], ident_h[:])
            w_t = {}
            for nm, hh in w_h.items():
                w_t[nm] = const.tile(list(hh.shape), bf, name=f"wt_{nm}")
                nc.sync.dma_start(w_t[nm][:], hh[:])
            b_t = {}
            for nm, hh in bias_h.items():
                b_t[nm] = const.tile(list(hh.shape), f32, name=f"bt_{nm}")
                nc.sync.dma_start(b_t[nm][:], hh[:])
            h3_t = sm.tile([128, WN, d_out], f32)
            ssum_t = sm.tile([128, WN], f32)
            ex_t = sm.tile([128, d_out], f32)
            lns_t = sm.tile([128, WN], f32)
            fin_state = {"ndone": 0, "gfin": 0}

            def layer(L, tbl_in, xt_in, xt_out, wo, wr, bname, dd):
                last = L == 3
                pend = None  # (w, g, j, aggT_s)
                ccq = []   # groups whose collective slice is pending dispatch
                expq = []  # windows awaiting their Exp pass (layer 3)

                def dispatch_cc(g):
                    tbl_next = tbl2 if L == 1 else tbl3
                    nc.gpsimd.collective_compute(
                        "AllGather", mybir.AluOpType.bypass,
                        replica_groups=[list(range(NCORES))],
                        ins=[hbg[(L, g)][:, :]],
                        outs=[tbl_next[g * GROUP_ROWS:(g + 1) * GROUP_ROWS, :]],
                    )

                def flush_exp(final=False):
                    # per-group log-softmax finale: exp+accum each pending
                    # window, then Ln/subtract/store for complete groups
                    while expq:
                        w = expq.pop(0)
                        nc.scalar.activation(ex_t[:], h3_t[:, w, :], AF.Exp,
                                             accum_out=ssum_t[:, w:w + 1])
                        fin_state["ndone"] = w + 1
                    gdone = fin_state["ndone"] // G if not final else NG
                    while fin_state["gfin"] < gdone:
                        gg = fin_state["gfin"]
                        w0, w1 = gg * G, (gg + 1) * G
                        nc.scalar.activation(lns_t[:, w0:w1],
                                             ssum_t[:, w0:w1], AF.Ln)
                        nc.vector.tensor_tensor(
                            h3_t[:, w0:w1, :], h3_t[:, w0:w1, :],
                            lns_t[:, w0:w1].unsqueeze(2).broadcast_to(
                                [128, G, d_out]),
                            OP.subtract)
                        nc.sync.dma_start(
                            out_h[:].rearrange("(w d) n -> d w n", d=128)
                            [:, w0:w1, :],
                            h3_t[:, w0:w1, :])
                        fin_state["gfin"] = gg + 1

                def dense(w, g, j, aggT_s):
                    hp = ps.tile([128, dd], f32, tag="hp")
                    nc.tensor.matmul(hp[:], aggT_s[:], wo[:], start=True, stop=False)
                    nc.tensor.matmul(hp[:], xt_in[:, w * 128:(w + 1) * 128],
                                     wr[:], start=False, stop=True)
                    if bname is not None:
                        nc.vector.tensor_add(hp[:], hp[:], b_t[bname][:, 0:dd])
                    if not last:
                        nc.scalar.activation(
                            xn[:, w * 128:(w + 1) * 128], hp[:], AF.Relu)
                        nc.sync.dma_start(
                            hbg[(L, g)][j * 128:(j + 1) * 128, :],
                            xn[:, w * 128:(w + 1) * 128])
                        htp = ps.tile([128, 128], f32, tag="htp")
                        nc.tensor.matmul(htp[:], wo[:], aggT_s[:],
                                         start=True, stop=False)
                        nc.tensor.matmul(htp[:], wr[:],
                                         xt_in[:, w * 128:(w + 1) * 128],
                                         start=False, stop=True)
                        if bname is not None:
                            nc.scalar.activation(
                                xt_out[:, w * 128:(w + 1) * 128], htp[:],
                                AF.Relu, bias=b_t[bname + "c"][:, 0:1])
                        else:
                            nc.scalar.activation(
                                xt_out[:, w * 128:(w + 1) * 128], htp[:], AF.Relu)
                    else:
                        nc.scalar.activation(
                            h3_t[:, w, :], hp[:], AF.Relu)
                        expq.append(w)
                    if not last and j == G - 1:
                        ccq.append(g)

                for g in range(NG):
                    msg = msgp.tile([128, G * CH, d_h], bf, tag="msg")
                    col0 = (g * CH * 8) * G
                    nlo16, nhi16 = G * NL * 8, G * NH * 8
                    nq = int(os.environ.get("GCN_NQ", "4"))
                    if g == 0 and nq >= 4:
                        # fill the pipe fast at layer start: split the first
                        # group's gathers across all queues
                        nA = (G * NL) // 2
                        nB = G * NL - nA
                        nc.gpsimd.dma_gather(
                            msg[:, 0:nA, :], tbl_in[0:HALF_T, :],
                            gidx_t[:, col0:col0 + nA * 8],
                            nA * 128, nA * 128, d_h,
                            single_packet=False, queue_num=0)
                        nc.gpsimd.dma_gather(
                            msg[:, nA:G * NL, :], tbl_in[0:HALF_T, :],
                            gidx_t[:, col0 + nA * 8:col0 + nlo16],
                            nB * 128, nB * 128, d_h,
                            single_packet=False, queue_num=2)
                        mA = (G * NH) // 2
                        mB = G * NH - mA
                        nc.gpsimd.dma_gather(
                            msg[:, G * NL:G * NL + mA, :], tbl_in[HALF_T:NT, :],
                            gidx_t[:, col0 + nlo16:col0 + nlo16 + mA * 8],
                            mA * 128, mA * 128, d_h,
                            single_packet=False, queue_num=1)
                        nc.gpsimd.dma_gather(
                            msg[:, G * NL + mA:G * CH, :], tbl_in[HALF_T:NT, :],
                            gidx_t[:, col0 + nlo16 + mA * 8:col0 + nlo16 + nhi16],
                            mB * 128, mB * 128, d_h,
                            single_packet=False, queue_num=3)
                    else:
                        nc.gpsimd.dma_gather(
                            msg[:, 0:G * NL, :], tbl_in[0:HALF_T, :],
                            gidx_t[:, col0:col0 + nlo16],
                            G * NL * 128, G * NL * 128, d_h,
                            single_packet=False,
                            queue_num=(2 * g) % nq,
                        )
                        nc.gpsimd.dma_gather(
                            msg[:, G * NL:G * CH, :], tbl_in[HALF_T:NT, :],
                            gidx_t[:, col0 + nlo16:col0 + nlo16 + nhi16],
                            G * NH * 128, G * NH * 128, d_h,
                            single_packet=False,
                            queue_num=(2 * g + 1) % nq,
                        )
                    s_t = sp.tile([128, G * CH * 128], f8, tag="s_t")
                    nc.sync.dma_start(
                        s_t[:], sh_h[:, g * G * CH * 128:(g + 1) * G * CH * 128])
                    while ccq and ccq[0] <= g - 2:
                        dispatch_cc(ccq.pop(0))
                    if last:
                        flush_exp()
                    for j in range(G):
                        w = g * G + j
                        aggT = ps.tile([128, 128], f32, tag="aggT")
                        # self-loop term: aggT[f,d] = xn[d,f] via identity
                        nc.tensor.matmul(
                            aggT[:], xn[:, w * 128:(w + 1) * 128], ident_t[:],
                            start=True, stop=False,
                        )
                        for c in range(CH):
                            pos = j * NL + c if c < NL else G * NL + j * NH + (c - NL)
                            nc.tensor.matmul(
                                aggT[:], msg[:, pos, :],
                                s_t[:, (j * CH + c) * 128:(j * CH + c + 1) * 128],
                                start=False, stop=(c == CH - 1),
                            )
                        aggT_s = wk.tile([128, 128], bf, tag="aggs")
                        nc.vector.tensor_tensor(
                            aggT_s[:], aggT[:],
                            dinv_t[:, w * 128:(w + 1) * 128], OP.mult)
                        if pend is not None:
                            dense(*pend)
                        pend = (w, g, j, aggT_s)
                dense(*pend)
                while ccq:
                    dispatch_cc(ccq.pop(0))
                if last:
                    flush_exp(final=True)

            layer(1, tbl1, xta, xtb, w_t["w1o"], w_t["w1r"],
                  "b1" if use_bias else None, d_h)
            layer(2, tbl2, xtb, xta, w_t["w2o"], w_t["w2r"],
                  "b2" if use_bias else None, d_h)
            layer(3, tbl3, xta, None, w_t["w3o"], w_t["w3r"],
                  "b3" if use_bias else None, d_out)



    nc.compile()
    return nc


def kernel(x, edge_index, W1_out, b1, W1_root, W2_out, b2, W2_root,
           W3_out, b3, W3_root):
    global LAST_EXEC_NS
    x = np.asarray(x, np.float32)
    edge_index = np.asarray(edge_index)
    d_in = x.shape[1]
    d_h = W1_out.shape[1]
    d_out = W3_out.shape[1]
    per_core, tbl1, dims = _preprocess(x, edge_index)
    use_bias = bool(np.any(b1) or np.any(b2) or np.any(b3))

    nc = _build(dims, d_in, d_h, d_out, use_bias)

    in_maps = []
    for k in range(NCORES):
        gidx, Sh, DINV = per_core[k]
        xt1 = np.zeros((128, ROWS), BF16)
        xt1[:, :C] = x[k * C:(k + 1) * C].T.astype(BF16)
        xn1 = np.zeros((128, ROWS), BF16)
        xk = x[k * C:(k + 1) * C].astype(BF16)          # [C, 128]
        xn1[:, :] = np.pad(xk, ((0, ROWS - C), (0, 0))).reshape(
            WN, 128, 128).transpose(1, 0, 2).reshape(128, ROWS)
        m = {
            "tbl1": tbl1,
            "gidx": gidx, "sh": Sh, "dinv": DINV, "xt1": xt1,
            "xn1": xn1, "ident": IDENT,
            "w1o": np.asarray(W1_out, np.float32).astype(BF16),
            "w1r": np.asarray(W1_root, np.float32).astype(BF16),
            "w2o": np.asarray(W2_out, np.float32).astype(BF16),
            "w2r": np.asarray(W2_root, np.float32).astype(BF16),
            "w3o": np.asarray(W3_out, np.float32).astype(BF16),
            "w3r": np.asarray(W3_root, np.float32).astype(BF16),
        }
        if use_bias:
            m["b1"] = np.tile(np.asarray(b1, np.float32), (128, 1))
            m["b2"] = np.tile(np.asarray(b2, np.float32), (128, 1))
            m["b3"] = np.tile(np.asarray(b3, np.float32), (128, 1))
            m["b1c"] = np.asarray(b1, np.float32).reshape(128, 1)
            m["b2c"] = np.asarray(b2, np.float32).reshape(128, 1)
        in_maps.append(m)

    trace = bool(int(os.environ.get("BASS_GCN_TRACE", "1")))
    if trace:
        try:
            # NTFF profiling under axon needs this hook module; without it
            # run_bass_kernel_spmd(trace=True) raises instead of degrading.
            from antenv.axon_hooks import get_axon_ntff_profile_hook  # noqa
        except ImportError:
            trace = False
    try:
        res = run_bass_kernel_spmd(nc, in_maps, core_ids=list(range(NCORES)),
                                   trace=trace)
    except Exception:
        if not trace:
            raise
        res = run_bass_kernel_spmd(nc, in_maps, core_ids=list(range(NCORES)),
                                   trace=False)
    LAST_EXEC_NS = res.exec_time_ns
    out = np.concatenate([res.results[k]["out"][:C] for k in range(NCORES)], axis=0)
    return out.astype(np.float32)
